# revision 19
# baseline (speedup 1.0000x reference)
"""ARAP gradient kernel for 8 TRN2 NeuronCores — SBUF-table single-pass gather.

Vertex-sharded: core r owns vertices [12500r, 12500(r+1)) for all 8 batches.
Each core builds a bf16 feature table slice [12544 slots, 128] (8 batches x 16
features: p1(3), t=R@p1-2*p2(3), R(9), const 1), AllGathers it (100352 rows),
and loads the full table into SBUF transposed: token t -> partition t%128,
256B row at free offset (t//128)*256.

Phase C gathers each edge's neighbor row exactly once with SBUF-source
dma_gather (transpose=True, 512-idx calls), edges pre-sorted by (chunk of
neighbor token, source slot). Ragged per-vertex segments are reduced on the
PE: per 128-edge block, transpose the gathered [128f, 128e] tile to edge-major
and matmul with an on-chip-built staircase weight matrix X[e, v] = w_e *
(vcol_e == v), accumulating in PSUM per (chunk, 128-slot window). Partials
S_part[c] are summed in Phase D, which then applies the per-vertex combine:

  g_i = aw * (2*W*p2_i - R_i(W*p1_i - S1_i) - SR_i*p1_i + St_i)
"""

import numpy as np

B = 8
N = 100000
K = 16
NCORES = 8
VREAL = N // NCORES          # 12500
VPC = 12544                  # 128*98 = 16*784
QCOL = 98
RANKS = 784
TROWS = NCORES * VPC         # 100352 = 784*128
F = 128
NCH = 4
CHR = RANKS // NCH           # 196 ranks per chunk window
CHTOK = CHR * 128            # 25088 tokens per chunk
NW = VPC // 128              # 98 slot windows
CALL = 896                   # idx per dma_gather call (1024 crashes HW)
GTE = 1792                   # edges per G tile (2 calls)
CPT = GTE // CALL            # calls per G tile
BPT = GTE // 128             # 128-edge blocks per G tile

# reserved all-zero slots, one per chunk (chunk of slot s = (s % 784)//196)
ZSLOT = [195, 391, 587, 12543]
_slots = np.setdiff1d(np.arange(VPC), ZSLOT)
SLOT_OF = _slots[:VREAL].copy()          # local vertex id -> slot
PAD_SLOTS = np.concatenate([ZSLOT, _slots[VREAL:]])

_cache = {}


def _token_of_global_row(g):
    return (g % RANKS) * 128 + g // RANKS


ZT_REL = []
for _c in range(NCH):
    _t = _token_of_global_row(ZSLOT[_c])
    assert _t // CHTOK == _c, (_c, _t)
    ZT_REL.append(_t - CHTOK * _c)


def _build(nt_per_chunk, blocks_cw):
    """nt_per_chunk: tuple of 4 ints (G tiles per chunk).
    blocks_cw: tuple of 4 tuples of 98 ints (128-edge blocks per cell)."""
    from concourse import bass, bacc, mybir
    from concourse.tile import TileContext

    nc = bacc.Bacc(None, num_swdge_queues=4)
    dt = mybir.dt

    NT_TOT = sum(nt_per_chunk)
    NBLK_TOT = BPT * NT_TOT

    xyz1_p = nc.declare_dram_parameter("xyz1s", [B, VPC, 3], dt.float32, isOutput=False)
    xyz1d_p = nc.declare_dram_parameter("xyz1d", [B, VPC, 3], dt.float32, isOutput=False)
    xyz2d_p = nc.declare_dram_parameter("xyz2d", [B, VPC, 3], dt.float32, isOutput=False)
    rotd_p = nc.declare_dram_parameter("rotsd", [B, VPC, 9], dt.float32, isOutput=False)
    xyz2_p = nc.declare_dram_parameter("xyz2s", [B, VPC, 3], dt.float32, isOutput=False)
    rot_p = nc.declare_dram_parameter("rots", [B, VPC, 9], dt.float32, isOutput=False)
    idx_p = nc.declare_dram_parameter("idxw", [NT_TOT, CPT, 128, CALL // 16], dt.int16, isOutput=False)
    xr_p = nc.declare_dram_parameter("xraw", [NBLK_TOT, 128, 2], dt.bfloat16, isOutput=False)
    id_p = nc.declare_dram_parameter("ident", [128, 128], dt.bfloat16, isOutput=False)
    msk_p = nc.declare_dram_parameter("smask", [128, QCOL], dt.bfloat16, isOutput=False)
    aw_p = nc.declare_dram_parameter("aw", [128, 1], dt.float32, isOutput=False)
    g_p = nc.declare_dram_parameter("g", [B, VPC, 3], dt.float32, isOutput=True)

    myT = nc.dram_tensor("myT", [VPC, F], dt.bfloat16)
    T_all = nc.dram_tensor("T_all", [TROWS, F], dt.bfloat16, addr_space="Shared")
    S_part = nc.dram_tensor("S_part", [NCH, VPC, F], dt.float32)

    with TileContext(nc) as tc:
        with tc.tile_pool(name="cst", bufs=1) as cpool:
            aw_t = cpool.tile([128, 1], dt.float32, tag="aw")
            nc.sync.dma_start(out=aw_t[:], in_=aw_p[:, :])
            ident = cpool.tile([128, 128], dt.bfloat16, tag="ident")
            nc.sync.dma_start(out=ident[:], in_=id_p[:, :])
            iota_t = cpool.tile([128, 128], dt.bfloat16, tag="iota")
            nc.gpsimd.iota(iota_t[:], pattern=[[1, 128]], base=0,
                           channel_multiplier=0, allow_small_or_imprecise_dtypes=True)

            # ---- Phase A: feature table slice ---------------------------
            with tc.tile_pool(name="pha", bufs=2) as pool:
                FS = pool.tile([128, QCOL, F], dt.bfloat16, tag="FS")
                nc.vector.memset(FS[:, :, :], 0.0)
                smask = pool.tile([128, QCOL], dt.bfloat16, tag="smask")
                nc.sync.dma_start(out=smask[:], in_=msk_p[:, :])
                for b in range(B):
                    p1 = pool.tile([128, QCOL, 3], dt.float32, tag="p1")
                    p2 = pool.tile([128, QCOL, 3], dt.float32, tag="p2")
                    R = pool.tile([128, QCOL, 9], dt.float32, tag="R")
                    nc.sync.dma_start(out=p1[:], in_=xyz1_p[b].rearrange("(p q) c -> p q c", p=128))
                    nc.sync.dma_start(out=p2[:], in_=xyz2_p[b].rearrange("(p q) c -> p q c", p=128))
                    nc.sync.dma_start(out=R[:], in_=rot_p[b].rearrange("(p q) c -> p q c", p=128))
                    fo = b * 16
                    nc.vector.tensor_copy(out=FS[:, :, fo + 0 : fo + 3], in_=p1[:, :, :])
                    for a in range(3):
                        acc = pool.tile([128, QCOL], dt.float32, tag="acc")
                        tmp = pool.tile([128, QCOL], dt.float32, tag="tmp")
                        nc.vector.tensor_tensor(out=acc[:], in0=R[:, :, 3 * a], in1=p1[:, :, 0], op=mybir.AluOpType.mult)
                        nc.vector.tensor_tensor(out=tmp[:], in0=R[:, :, 3 * a + 1], in1=p1[:, :, 1], op=mybir.AluOpType.mult)
                        nc.vector.tensor_tensor(out=acc[:], in0=acc[:], in1=tmp[:], op=mybir.AluOpType.add)
                        nc.vector.tensor_tensor(out=tmp[:], in0=R[:, :, 3 * a + 2], in1=p1[:, :, 2], op=mybir.AluOpType.mult)
                        nc.vector.tensor_tensor(out=acc[:], in0=acc[:], in1=tmp[:], op=mybir.AluOpType.add)
                        nc.vector.tensor_scalar_mul(out=tmp[:], in0=p2[:, :, a], scalar1=-2.0)
                        nc.vector.tensor_tensor(out=FS[:, :, fo + 3 + a], in0=acc[:], in1=tmp[:], op=mybir.AluOpType.add)
                    nc.vector.tensor_copy(out=FS[:, :, fo + 6 : fo + 15], in_=R[:, :, :])
                    nc.vector.tensor_copy(out=FS[:, :, fo + 15], in_=smask[:])
                nc.sync.dma_start(out=myT.rearrange("(p q) f -> p q f", p=128), in_=FS[:, :, :])

            # ---- Phase B: AllGather -------------------------------------
            nc.gpsimd.collective_compute(
                "AllGather",
                mybir.AluOpType.bypass,
                replica_groups=[list(range(NCORES))],
                ins=[myT[:]],
                outs=[T_all[:]],
            )

            # ---- Phase C: single-pass gather + matmul segment reduce ----
            with (
                tc.tile_pool(name="tab", bufs=1) as tpool,
                tc.tile_pool(name="gat", bufs=2) as gpool,
                tc.tile_pool(name="blk", bufs=4) as bpool,
                tc.tile_pool(name="ssp", bufs=1) as sspool,
                tc.tile_pool(name="ps", bufs=2, space="PSUM") as ppool,
                tc.tile_pool(name="pst", bufs=4, space="PSUM") as ptpool,
            ):
                TB = tpool.tile([128, RANKS, F], dt.bfloat16, tag="TB")
                TAv = T_all.rearrange("(p s) f -> p s f", p=128)
                for cc in range(NCH):
                    nc.sync.dma_start(out=TB[:, CHR * cc : CHR * (cc + 1), :],
                                      in_=TAv[:, CHR * cc : CHR * (cc + 1), :])

                t_base = 0
                for c in range(NCH):
                    # block -> (window, start, stop) map for this chunk
                    wlist = []
                    for w in range(NW):
                        nb = blocks_cw[c][w]
                        for i in range(nb):
                            wlist.append((w, i == 0, i == nb - 1))
                    assert len(wlist) == BPT * nt_per_chunk[c]
                    Sacc = None
                    for t in range(nt_per_chunk[c]):
                        tg = t_base + t
                        G = gpool.tile([128, GTE], dt.bfloat16, name=f"G_{tg}", tag="G")
                        idx_t = gpool.tile([128, CPT, CALL // 16], dt.int16, name=f"ix_{tg}", tag="idx")
                        nc.sync.dma_start(out=idx_t[:], in_=idx_p[tg].rearrange("s p q -> p s q"))
                        for s in range(CPT):
                            nc.gpsimd.dma_gather(
                                out_ap=G[:, None, CALL * s : CALL * (s + 1)],
                                in_ap=TB[:, CHR * c : CHR * (c + 1), :],
                                idxs_ap=idx_t[:, s, :],
                                num_idxs=CALL,
                                num_idxs_reg=CALL,
                                elem_size=F,
                                transpose=True,
                                sbuf_tokens_per_rank=128,
                                sbuf_free_dim_per_rank=256,
                                queue_num=(CPT * tg + s) % 4,
                            )
                        Xr = gpool.tile([128, BPT, 2], dt.bfloat16, name=f"xr_{tg}", tag="Xr")
                        nc.sync.dma_start(out=Xr[:], in_=xr_p[BPT * tg : BPT * (tg + 1)].rearrange("b p c -> p b c"))
                        for bi in range(BPT):
                            w, is_start, is_stop = wlist[BPT * t + bi]
                            T1 = ptpool.tile([128, 128], dt.bfloat16, name=f"T1_{tg}_{bi}", tag="T1")
                            nc.tensor.transpose(T1[:], G[:, 128 * bi : 128 * (bi + 1)], ident[:])
                            Gt = bpool.tile([128, 128], dt.bfloat16, name=f"Gt_{tg}_{bi}", tag="Gt")
                            nc.scalar.copy(out=Gt[:], in_=T1[:])
                            X = bpool.tile([128, 128], dt.bfloat16, name=f"X_{tg}_{bi}", tag="X")
                            nc.vector.tensor_tensor(
                                out=X[:], in0=Xr[:, bi, 0:1].to_broadcast([128, 128]),
                                in1=iota_t[:], op=mybir.AluOpType.is_equal)
                            nc.vector.tensor_tensor(
                                out=X[:], in0=X[:],
                                in1=Xr[:, bi, 1:2].to_broadcast([128, 128]),
                                op=mybir.AluOpType.mult)
                            if is_start:
                                Sacc = ppool.tile([128, 128], dt.float32, name=f"Sa_{c}_{w}", tag="Sacc")
                            nc.tensor.matmul(out=Sacc[:], lhsT=X[:], rhs=Gt[:], start=is_start, stop=is_stop)
                            if is_stop:
                                Ss = sspool.tile([128, 128], dt.float32, name=f"Ss_{c}_{w}", tag="Ss")
                                nc.scalar.copy(out=Ss[:], in_=Sacc[:])
                                nc.sync.dma_start(out=S_part[c, 128 * w : 128 * (w + 1), :], in_=Ss[:])
                    t_base += nt_per_chunk[c]

            # ---- Phase D: combine ---------------------------------------
            with tc.tile_pool(name="phd", bufs=2) as pool, tc.tile_pool(name="phs", bufs=1) as spool:
                S = spool.tile([128, QCOL, F], dt.float32, tag="S")
                nc.sync.dma_start(out=S[:], in_=S_part[0].rearrange("(p q) f -> p q f", p=128))
                for c in range(1, NCH):
                    Stmp = spool.tile([128, QCOL, F], dt.float32, name=f"Stmp{c}", tag=f"Stmp{c % 2}")
                    nc.sync.dma_start(out=Stmp[:], in_=S_part[c].rearrange("(p q) f -> p q f", p=128))
                    nc.vector.tensor_tensor(out=S[:], in0=S[:], in1=Stmp[:], op=mybir.AluOpType.add)
                awb = aw_t[:, :].to_broadcast([128, QCOL])
                for b in range(B):
                    p1 = pool.tile([128, QCOL, 3], dt.float32, tag="p1")
                    p2 = pool.tile([128, QCOL, 3], dt.float32, tag="p2")
                    R = pool.tile([128, QCOL, 9], dt.float32, tag="R")
                    nc.sync.dma_start(out=p1[:], in_=xyz1d_p[b].rearrange("(p q) c -> p q c", p=128))
                    nc.sync.dma_start(out=p2[:], in_=xyz2d_p[b].rearrange("(p q) c -> p q c", p=128))
                    nc.sync.dma_start(out=R[:], in_=rotd_p[b].rearrange("(p q) c -> p q c", p=128))
                    fo = b * 16
                    W = S[:, :, fo + 15]
                    gout = pool.tile([128, QCOL, 3], dt.float32, tag="gout")
                    u = pool.tile([128, QCOL, 3], dt.float32, tag="u")
                    for a in range(3):
                        tmp = pool.tile([128, QCOL], dt.float32, tag="tmp")
                        nc.vector.tensor_tensor(out=tmp[:], in0=W, in1=p1[:, :, a], op=mybir.AluOpType.mult)
                        nc.vector.tensor_tensor(out=u[:, :, a], in0=tmp[:], in1=S[:, :, fo + a], op=mybir.AluOpType.subtract)
                    for a in range(3):
                        acc = pool.tile([128, QCOL], dt.float32, tag="acc")
                        tmp = pool.tile([128, QCOL], dt.float32, tag="tmp")
                        nc.vector.tensor_tensor(out=acc[:], in0=R[:, :, 3 * a], in1=u[:, :, 0], op=mybir.AluOpType.mult)
                        nc.vector.tensor_tensor(out=tmp[:], in0=R[:, :, 3 * a + 1], in1=u[:, :, 1], op=mybir.AluOpType.mult)
                        nc.vector.tensor_tensor(out=acc[:], in0=acc[:], in1=tmp[:], op=mybir.AluOpType.add)
                        nc.vector.tensor_tensor(out=tmp[:], in0=R[:, :, 3 * a + 2], in1=u[:, :, 2], op=mybir.AluOpType.mult)
                        nc.vector.tensor_tensor(out=acc[:], in0=acc[:], in1=tmp[:], op=mybir.AluOpType.add)
                        for j in range(3):
                            nc.gpsimd.tensor_tensor(out=tmp[:], in0=S[:, :, fo + 6 + 3 * a + j], in1=p1[:, :, j], op=mybir.AluOpType.mult)
                            nc.vector.tensor_tensor(out=acc[:], in0=acc[:], in1=tmp[:], op=mybir.AluOpType.add)
                        nc.gpsimd.tensor_tensor(out=tmp[:], in0=W, in1=p2[:, :, a], op=mybir.AluOpType.mult)
                        nc.vector.tensor_scalar_mul(out=tmp[:], in0=tmp[:], scalar1=2.0)
                        nc.vector.tensor_tensor(out=tmp[:], in0=tmp[:], in1=acc[:], op=mybir.AluOpType.subtract)
                        nc.vector.tensor_tensor(out=tmp[:], in0=tmp[:], in1=S[:, :, fo + 3 + a], op=mybir.AluOpType.add)
                        nc.vector.tensor_tensor(out=gout[:, :, a], in0=tmp[:], in1=awb, op=mybir.AluOpType.mult)
                    nc.sync.dma_start(out=g_p[b].rearrange("(p q) c -> p q c", p=128), in_=gout[:])

    nc.compile()
    return nc


def _balance(cnt):
    """Assign vertices to 98 windows of 128 positions, balancing each window's
    per-chunk edge counts toward multiples of 128. Returns dpos[l] in [0, VPC)."""
    key = cnt @ np.array([1, 17, 17 ** 2, 17 ** 3])
    ukeys, inv = np.unique(key, return_inverse=True)
    T = len(ukeys)
    tvec = np.zeros((T, 4), np.int64)
    tvec[inv] = cnt
    avail = np.bincount(inv, minlength=T).astype(np.int64)
    order_by_type = np.argsort(inv, kind="stable")
    ptr = np.cumsum(np.bincount(inv, minlength=T))
    tv = tvec.astype(np.float64)
    dpos = np.empty(VREAL, np.int64)
    rem = VREAL
    for w in range(NW):
        npick = min(128, rem)
        if npick == 0:
            break
        t = np.full(4, npick * 4.0)
        got = np.zeros(4)
        for i in range(npick):
            m = npick - i
            ideal = (t - got) / m
            d = ((tv - ideal) ** 2).sum(1)
            d[avail == 0] = 1e18
            j = int(np.argmin(d))
            avail[j] -= 1
            ptr[j] -= 1
            dpos[order_by_type[ptr[j]]] = w * 128 + i
            got += tvec[j]
        rem -= npick
    return dpos


def _host_prep(xyz1, xyz2, neighborList, numNeighbors, accnumNeighbors,
               weightMatrix, rotations, arapWeight):
    nbr = np.asarray(neighborList).astype(np.int64)
    wm = np.asarray(weightMatrix).astype(np.float32)
    import jax.numpy as jnp

    def bf16(x):
        return np.asarray(jnp.asarray(x, jnp.bfloat16))

    # neighbor global row / token / chunk (vertex -> slot is core-independent)
    g_of_v = 12544 * (np.arange(N) // VREAL) + SLOT_OF[np.arange(N) % VREAL]
    tok_of_v = _token_of_global_row(g_of_v)

    # per-core edge arrays in (slot, k) order
    l_e = np.repeat(np.arange(VREAL), K)
    k_e = np.tile(np.arange(K), VREAL)
    s_e = SLOT_OF[l_e]

    cores = []
    dpos_list = []
    counts_all = np.zeros((NCORES, NCH, NW), np.int64)
    for r in range(NCORES):
        e_id = (VREAL * r + l_e) * K + k_e
        tok = tok_of_v[nbr[e_id]]
        ch = tok // CHTOK
        rel = tok - CHTOK * ch
        w_e = wm[e_id]
        cnt = np.zeros((VREAL, NCH), np.int64)
        np.add.at(cnt, (l_e, ch), 1)
        dpos = _balance(cnt)
        dpos_list.append(dpos)
        d_e = dpos[l_e]
        order = np.lexsort((k_e, d_e, ch))
        cores.append((ch[order], rel[order], d_e[order], w_e[order]))
        counts_all[r] = np.bincount(ch[order] * NW + d_e[order] // 128,
                                    minlength=NCH * NW).reshape(NCH, NW)

    caps = counts_all.max(axis=0)                       # [NCH, NW]
    blocks_cw = np.maximum((caps + 127) // 128, 1)      # >=1 block per cell
    nt_per_chunk = []
    for c in range(NCH):
        tot = int(blocks_cw[c].sum())
        extra = (-tot) % BPT                            # pad chunk to G-tile mult
        blocks_cw[c, NW - 1] += extra
        nt_per_chunk.append((tot + extra) // BPT)
    nt_per_chunk = tuple(nt_per_chunk)
    blocks_key = tuple(tuple(int(x) for x in blocks_cw[c]) for c in range(NCH))

    NT_TOT = sum(nt_per_chunk)
    NBLK_TOT = BPT * NT_TOT

    # cell start offsets in each chunk stream (in edges)
    cell_off = np.zeros((NCH, NW), np.int64)
    for c in range(NCH):
        cell_off[c] = np.concatenate([[0], np.cumsum(blocks_cw[c][:-1] * 128)])
    chunk_len = [int(blocks_cw[c].sum() * 128) for c in range(NCH)]
    chunk_tile_base = np.concatenate([[0], np.cumsum(nt_per_chunk)]).astype(np.int64)

    in_maps = []
    ident = np.eye(128, dtype=np.float32)
    smask_host = np.zeros(VPC, np.float32)
    smask_host[SLOT_OF] = 1.0
    smask_host = smask_host.reshape(128, QCOL)
    for r in range(NCORES):
        ch_s, rel_s, s_s, w_s = cores[r]
        idx_stream = [np.full(chunk_len[c], ZT_REL[c], np.int16) for c in range(NCH)]
        vcol_stream = [np.zeros(chunk_len[c], np.float32) for c in range(NCH)]
        wgt_stream = [np.zeros(chunk_len[c], np.float32) for c in range(NCH)]
        # place each cell's edges at its stream offset
        pos_in_cell = np.zeros(NCH * NW, np.int64)
        cell_id = ch_s * NW + s_s // 128
        # edges are sorted by (ch, slot, k) so within each cell they are in order
        # compute position of each edge within its cell:
        srt = np.argsort(cell_id, kind="stable")
        cid_sorted = cell_id[srt]
        first = np.concatenate([[0], np.cumsum(np.bincount(cid_sorted, minlength=NCH * NW))[:-1]])
        pos_sorted = np.arange(len(cid_sorted)) - first[cid_sorted]
        pos = np.empty_like(pos_sorted)
        pos[srt] = pos_sorted
        for c in range(NCH):
            m = ch_s == c
            p = cell_off[c][s_s[m] // 128] + pos[m]
            idx_stream[c][p] = rel_s[m].astype(np.int16)
            vcol_stream[c][p] = (s_s[m] % 128).astype(np.float32)
            wgt_stream[c][p] = w_s[m]

        idxw = np.zeros((NT_TOT, CPT, 128, CALL // 16), np.int16)
        xraw = np.zeros((NBLK_TOT, 128, 2), np.float32)
        for c in range(NCH):
            st = idx_stream[c].reshape(-1, CPT, CALL // 16, 16)  # [tiles, call, f, l]
            idxw[chunk_tile_base[c] : chunk_tile_base[c + 1]] = np.tile(
                np.transpose(st, (0, 1, 3, 2)), (1, 1, 8, 1))
            b0 = BPT * chunk_tile_base[c]
            nb = chunk_len[c] // 128
            xraw[b0 : b0 + nb, :, 0] = vcol_stream[c].reshape(nb, 128)
            xraw[b0 : b0 + nb, :, 1] = wgt_stream[c].reshape(nb, 128)

        in_maps.append({
            "xyz1s": None, "xyz2s": None, "rots": None,
            "idxw": idxw, "xraw": bf16(xraw), "ident": bf16(ident),
            "smask": bf16(smask_host),
            "aw": np.full((128, 1), np.float32(arapWeight)),
        })

    # permuted per-core xyz/rot inputs (slot layout, pads zero)
    xyz1 = np.asarray(xyz1)
    xyz2 = np.asarray(xyz2)
    rots = np.asarray(rotations).reshape(B, N, 9)
    for r in range(NCORES):
        x1 = np.zeros((B, VPC, 3), np.float32)
        x2 = np.zeros((B, VPC, 3), np.float32)
        rr = np.zeros((B, VPC, 9), np.float32)
        v0 = r * VREAL
        x1[:, SLOT_OF] = xyz1[:, v0 : v0 + VREAL]
        x2[:, SLOT_OF] = xyz2[:, v0 : v0 + VREAL]
        rr[:, SLOT_OF] = rots[:, v0 : v0 + VREAL]
        in_maps[r]["xyz1s"] = x1
        in_maps[r]["xyz2s"] = x2
        in_maps[r]["rots"] = rr
        x1d = np.zeros((B, VPC, 3), np.float32)
        x2d = np.zeros((B, VPC, 3), np.float32)
        rrd = np.zeros((B, VPC, 9), np.float32)
        x1d[:, dpos_list[r]] = xyz1[:, v0 : v0 + VREAL]
        x2d[:, dpos_list[r]] = xyz2[:, v0 : v0 + VREAL]
        rrd[:, dpos_list[r]] = rots[:, v0 : v0 + VREAL]
        in_maps[r]["xyz1d"] = x1d
        in_maps[r]["xyz2d"] = x2d
        in_maps[r]["rotsd"] = rrd

    return in_maps, nt_per_chunk, blocks_key, dpos_list


def _execute(in_maps, nt_per_chunk, blocks_key, trace=False, **kw):
    from concourse.bass_utils import run_bass_kernel_spmd
    key = (nt_per_chunk, blocks_key)
    if key not in _cache:
        _cache[key] = _build(nt_per_chunk, blocks_key)
    return run_bass_kernel_spmd(_cache[key], in_maps, list(range(NCORES)), trace=trace, **kw)


def kernel(**inputs):
    in_maps, ntc, bkey, dpos_list = _host_prep(**inputs)
    res = _execute(in_maps, ntc, bkey)
    out = np.concatenate(
        [res.results[r]["g"][:, dpos_list[r]] for r in range(NCORES)], axis=1)
    return out.astype(np.float32)


# revision 20
# speedup vs baseline: 1.1936x; 1.1936x over previous
"""ARAP gradient kernel for 8 TRN2 NeuronCores — SBUF-table single-pass gather.

Vertex-sharded: core r owns vertices [12500r, 12500(r+1)) for all 8 batches.
Each core builds a bf16 feature table slice [12544 slots, 128] (8 batches x 16
features: p1(3), t=R@p1-2*p2(3), R(9), const 1), AllGathers it (100352 rows),
and loads the full table into SBUF transposed: token t -> partition t%128,
256B row at free offset (t//128)*256.

Phase C gathers each edge's neighbor row exactly once with SBUF-source
dma_gather (transpose=True, 512-idx calls), edges pre-sorted by (chunk of
neighbor token, source slot). Ragged per-vertex segments are reduced on the
PE: per 128-edge block, transpose the gathered [128f, 128e] tile to edge-major
and matmul with an on-chip-built staircase weight matrix X[e, v] = w_e *
(vcol_e == v), accumulating in PSUM per (chunk, 128-slot window). Partials
S_part[c] are summed in Phase D, which then applies the per-vertex combine:

  g_i = aw * (2*W*p2_i - R_i(W*p1_i - S1_i) - SR_i*p1_i + St_i)
"""

import numpy as np

B = 8
N = 100000
K = 16
NCORES = 8
VREAL = N // NCORES          # 12500
VPC = 12544                  # 128*98 = 16*784
QCOL = 98
RANKS = 784
TROWS = NCORES * VPC         # 100352 = 784*128
F = 128
NCH = 4
CHR = RANKS // NCH           # 196 ranks per chunk window
CHTOK = CHR * 128            # 25088 tokens per chunk
NW = VPC // 128              # 98 slot windows
CALL = 896                   # idx per dma_gather call (1024 crashes HW)
GTE = 1792                   # edges per G tile (2 calls)
CPT = GTE // CALL            # calls per G tile
BPT = GTE // 128             # 128-edge blocks per G tile

# reserved all-zero slots, one per chunk (chunk of slot s = (s % 784)//196)
ZSLOT = [195, 391, 587, 12543]
_slots = np.setdiff1d(np.arange(VPC), ZSLOT)
SLOT_OF = _slots[:VREAL].copy()          # local vertex id -> slot
PAD_SLOTS = np.concatenate([ZSLOT, _slots[VREAL:]])

_cache = {}


def _token_of_global_row(g):
    return (g % RANKS) * 128 + g // RANKS


ZT_REL = []
for _c in range(NCH):
    _t = _token_of_global_row(ZSLOT[_c])
    assert _t // CHTOK == _c, (_c, _t)
    ZT_REL.append(_t - CHTOK * _c)


def _build(nt_per_chunk, blocks_cw):
    """nt_per_chunk: tuple of 4 ints (G tiles per chunk).
    blocks_cw: tuple of 4 tuples of 98 ints (128-edge blocks per cell)."""
    from concourse import bass, bacc, mybir
    from concourse.tile import TileContext

    nc = bacc.Bacc(None, num_swdge_queues=4)
    dt = mybir.dt

    NT_TOT = sum(nt_per_chunk)
    NBLK_TOT = BPT * NT_TOT

    xyz1_p = nc.declare_dram_parameter("xyz1s", [B, VPC, 3], dt.float32, isOutput=False)
    xyz1d_p = nc.declare_dram_parameter("xyz1d", [B, VPC, 3], dt.float32, isOutput=False)
    xyz2d_p = nc.declare_dram_parameter("xyz2d", [B, VPC, 3], dt.float32, isOutput=False)
    rotd_p = nc.declare_dram_parameter("rotsd", [B, VPC, 9], dt.float32, isOutput=False)
    xyz2_p = nc.declare_dram_parameter("xyz2s", [B, VPC, 3], dt.float32, isOutput=False)
    rot_p = nc.declare_dram_parameter("rots", [B, VPC, 9], dt.float32, isOutput=False)
    idx_p = nc.declare_dram_parameter("idxw", [NT_TOT, CPT, 128, CALL // 16], dt.int16, isOutput=False)
    xr_p = nc.declare_dram_parameter("xraw", [NBLK_TOT, 128, 2], dt.bfloat16, isOutput=False)
    id_p = nc.declare_dram_parameter("ident", [128, 128], dt.bfloat16, isOutput=False)
    msk_p = nc.declare_dram_parameter("smask", [128, QCOL], dt.bfloat16, isOutput=False)
    aw_p = nc.declare_dram_parameter("aw", [128, 1], dt.float32, isOutput=False)
    g_p = nc.declare_dram_parameter("g", [B, VPC, 3], dt.float32, isOutput=True)

    myT = nc.dram_tensor("myT", [VPC, F], dt.bfloat16)
    T_all = nc.dram_tensor("T_all", [TROWS, F], dt.bfloat16, addr_space="Shared")
    S_part = nc.dram_tensor("S_part", [NCH, VPC, F], dt.float32)

    with TileContext(nc) as tc:
        with tc.tile_pool(name="cst", bufs=1) as cpool:
            aw_t = cpool.tile([128, 1], dt.float32, tag="aw")
            nc.sync.dma_start(out=aw_t[:], in_=aw_p[:, :])
            ident = cpool.tile([128, 128], dt.bfloat16, tag="ident")
            nc.sync.dma_start(out=ident[:], in_=id_p[:, :])
            iota_t = cpool.tile([128, 128], dt.bfloat16, tag="iota")
            nc.gpsimd.iota(iota_t[:], pattern=[[1, 128]], base=0,
                           channel_multiplier=0, allow_small_or_imprecise_dtypes=True)

            # ---- Phase A: feature table slice ---------------------------
            with tc.tile_pool(name="pha", bufs=2) as pool:
                FS = pool.tile([128, QCOL, F], dt.bfloat16, tag="FS")
                nc.vector.memset(FS[:, :, :], 0.0)
                smask = pool.tile([128, QCOL], dt.bfloat16, tag="smask")
                nc.sync.dma_start(out=smask[:], in_=msk_p[:, :])
                for b in range(B):
                    p1 = pool.tile([128, QCOL, 3], dt.float32, tag="p1")
                    p2 = pool.tile([128, QCOL, 3], dt.float32, tag="p2")
                    R = pool.tile([128, QCOL, 9], dt.float32, tag="R")
                    nc.sync.dma_start(out=p1[:], in_=xyz1_p[b].rearrange("(p q) c -> p q c", p=128))
                    nc.sync.dma_start(out=p2[:], in_=xyz2_p[b].rearrange("(p q) c -> p q c", p=128))
                    nc.sync.dma_start(out=R[:], in_=rot_p[b].rearrange("(p q) c -> p q c", p=128))
                    fo = b * 16
                    nc.vector.tensor_copy(out=FS[:, :, fo + 0 : fo + 3], in_=p1[:, :, :])
                    for a in range(3):
                        acc = pool.tile([128, QCOL], dt.float32, tag="acc")
                        tmp = pool.tile([128, QCOL], dt.float32, tag="tmp")
                        nc.vector.tensor_tensor(out=acc[:], in0=R[:, :, 3 * a], in1=p1[:, :, 0], op=mybir.AluOpType.mult)
                        nc.vector.tensor_tensor(out=tmp[:], in0=R[:, :, 3 * a + 1], in1=p1[:, :, 1], op=mybir.AluOpType.mult)
                        nc.vector.tensor_tensor(out=acc[:], in0=acc[:], in1=tmp[:], op=mybir.AluOpType.add)
                        nc.vector.tensor_tensor(out=tmp[:], in0=R[:, :, 3 * a + 2], in1=p1[:, :, 2], op=mybir.AluOpType.mult)
                        nc.vector.tensor_tensor(out=acc[:], in0=acc[:], in1=tmp[:], op=mybir.AluOpType.add)
                        nc.vector.tensor_scalar_mul(out=tmp[:], in0=p2[:, :, a], scalar1=-2.0)
                        nc.vector.tensor_tensor(out=FS[:, :, fo + 3 + a], in0=acc[:], in1=tmp[:], op=mybir.AluOpType.add)
                    nc.vector.tensor_copy(out=FS[:, :, fo + 6 : fo + 15], in_=R[:, :, :])
                    nc.vector.tensor_copy(out=FS[:, :, fo + 15], in_=smask[:])
                nc.sync.dma_start(out=myT.rearrange("(p q) f -> p q f", p=128), in_=FS[:, :, :])

            # ---- Phase B: AllGather -------------------------------------
            nc.gpsimd.collective_compute(
                "AllGather",
                mybir.AluOpType.bypass,
                replica_groups=[list(range(NCORES))],
                ins=[myT[:]],
                outs=[T_all[:]],
            )

            # ---- Phase C: single-pass gather + matmul segment reduce ----
            with (
                tc.tile_pool(name="tab", bufs=1) as tpool,
                tc.tile_pool(name="gat", bufs=2) as gpool,
                tc.tile_pool(name="blk", bufs=2) as bpool,
                tc.tile_pool(name="ps", bufs=2, space="PSUM") as ppool,
            ):
                TB = tpool.tile([128, RANKS, F], dt.bfloat16, tag="TB")
                TAv = T_all.rearrange("(p s) f -> p s f", p=128)
                for cc in range(NCH):
                    nc.sync.dma_start(out=TB[:, CHR * cc : CHR * (cc + 1), :],
                                      in_=TAv[:, CHR * cc : CHR * (cc + 1), :])

                t_base = 0
                for c in range(NCH):
                    # block -> (window, start, stop) map for this chunk
                    wlist = []
                    for w in range(NW):
                        nb = blocks_cw[c][w]
                        for i in range(nb):
                            wlist.append((w, i == 0, i == nb - 1))
                    assert len(wlist) == BPT * nt_per_chunk[c]
                    Sacc = None
                    for t in range(nt_per_chunk[c]):
                        tg = t_base + t
                        G = gpool.tile([128, GTE], dt.bfloat16, name=f"G_{tg}", tag="G")
                        idx_t = gpool.tile([128, CPT, CALL // 16], dt.int16, name=f"ix_{tg}", tag="idx")
                        nc.sync.dma_start(out=idx_t[:], in_=idx_p[tg].rearrange("s p q -> p s q"))
                        for s in range(CPT):
                            nc.gpsimd.dma_gather(
                                out_ap=G[:, None, CALL * s : CALL * (s + 1)],
                                in_ap=TB[:, CHR * c : CHR * (c + 1), :],
                                idxs_ap=idx_t[:, s, :],
                                num_idxs=CALL,
                                num_idxs_reg=CALL,
                                elem_size=F,
                                transpose=True,
                                sbuf_tokens_per_rank=128,
                                sbuf_free_dim_per_rank=256,
                                queue_num=(CPT * tg + s) % 4,
                            )
                        Xr = gpool.tile([128, BPT, 2], dt.bfloat16, name=f"xr_{tg}", tag="Xr")
                        nc.sync.dma_start(out=Xr[:], in_=xr_p[BPT * tg : BPT * (tg + 1)].rearrange("b p c -> p b c"))
                        for bi in range(BPT):
                            w, is_start, is_stop = wlist[BPT * t + bi]
                            T1 = ppool.tile([128, 128], dt.bfloat16, name=f"T1_{tg}_{bi}", tag="T1")
                            nc.tensor.transpose(T1[:], G[:, 128 * bi : 128 * (bi + 1)], ident[:])
                            Gt = bpool.tile([128, 128], dt.bfloat16, name=f"Gt_{tg}_{bi}", tag="Gt")
                            nc.scalar.copy(out=Gt[:], in_=T1[:])
                            X = bpool.tile([128, 128], dt.bfloat16, name=f"X_{tg}_{bi}", tag="X")
                            nc.vector.tensor_tensor(
                                out=X[:], in0=Xr[:, bi, 0:1].to_broadcast([128, 128]),
                                in1=iota_t[:], op=mybir.AluOpType.is_equal)
                            nc.vector.tensor_tensor(
                                out=X[:], in0=X[:],
                                in1=Xr[:, bi, 1:2].to_broadcast([128, 128]),
                                op=mybir.AluOpType.mult)
                            if is_start:
                                Sacc = ppool.tile([128, 128], dt.float32, name=f"Sa_{c}_{w}", tag="Sacc")
                            nc.tensor.matmul(out=Sacc[:], lhsT=X[:], rhs=Gt[:], start=is_start, stop=is_stop)
                            if is_stop:
                                Ss = bpool.tile([128, 128], dt.float32, name=f"Ss_{c}_{w}", tag="Ss")
                                nc.scalar.copy(out=Ss[:], in_=Sacc[:])
                                nc.sync.dma_start(out=S_part[c, 128 * w : 128 * (w + 1), :], in_=Ss[:])
                    t_base += nt_per_chunk[c]

            # ---- Phase D: combine ---------------------------------------
            with tc.tile_pool(name="phd", bufs=2) as pool, tc.tile_pool(name="phs", bufs=1) as spool:
                S = spool.tile([128, QCOL, F], dt.float32, tag="S")
                nc.sync.dma_start(out=S[:], in_=S_part[0].rearrange("(p q) f -> p q f", p=128))
                for c in range(1, NCH):
                    Stmp = spool.tile([128, QCOL, F], dt.float32, name=f"Stmp{c}", tag=f"Stmp{c % 2}")
                    nc.sync.dma_start(out=Stmp[:], in_=S_part[c].rearrange("(p q) f -> p q f", p=128))
                    nc.vector.tensor_tensor(out=S[:], in0=S[:], in1=Stmp[:], op=mybir.AluOpType.add)
                awb = aw_t[:, :].to_broadcast([128, QCOL])
                for b in range(B):
                    p1 = pool.tile([128, QCOL, 3], dt.float32, tag="p1")
                    p2 = pool.tile([128, QCOL, 3], dt.float32, tag="p2")
                    R = pool.tile([128, QCOL, 9], dt.float32, tag="R")
                    nc.sync.dma_start(out=p1[:], in_=xyz1d_p[b].rearrange("(p q) c -> p q c", p=128))
                    nc.sync.dma_start(out=p2[:], in_=xyz2d_p[b].rearrange("(p q) c -> p q c", p=128))
                    nc.sync.dma_start(out=R[:], in_=rotd_p[b].rearrange("(p q) c -> p q c", p=128))
                    fo = b * 16
                    W = S[:, :, fo + 15]
                    gout = pool.tile([128, QCOL, 3], dt.float32, tag="gout")
                    u = pool.tile([128, QCOL, 3], dt.float32, tag="u")
                    for a in range(3):
                        tmp = pool.tile([128, QCOL], dt.float32, tag="tmp")
                        nc.vector.tensor_tensor(out=tmp[:], in0=W, in1=p1[:, :, a], op=mybir.AluOpType.mult)
                        nc.vector.tensor_tensor(out=u[:, :, a], in0=tmp[:], in1=S[:, :, fo + a], op=mybir.AluOpType.subtract)
                    for a in range(3):
                        acc = pool.tile([128, QCOL], dt.float32, tag="acc")
                        tmp = pool.tile([128, QCOL], dt.float32, tag="tmp")
                        nc.vector.tensor_tensor(out=acc[:], in0=R[:, :, 3 * a], in1=u[:, :, 0], op=mybir.AluOpType.mult)
                        nc.vector.tensor_tensor(out=tmp[:], in0=R[:, :, 3 * a + 1], in1=u[:, :, 1], op=mybir.AluOpType.mult)
                        nc.vector.tensor_tensor(out=acc[:], in0=acc[:], in1=tmp[:], op=mybir.AluOpType.add)
                        nc.vector.tensor_tensor(out=tmp[:], in0=R[:, :, 3 * a + 2], in1=u[:, :, 2], op=mybir.AluOpType.mult)
                        nc.vector.tensor_tensor(out=acc[:], in0=acc[:], in1=tmp[:], op=mybir.AluOpType.add)
                        for j in range(3):
                            nc.gpsimd.tensor_tensor(out=tmp[:], in0=S[:, :, fo + 6 + 3 * a + j], in1=p1[:, :, j], op=mybir.AluOpType.mult)
                            nc.vector.tensor_tensor(out=acc[:], in0=acc[:], in1=tmp[:], op=mybir.AluOpType.add)
                        nc.gpsimd.tensor_tensor(out=tmp[:], in0=W, in1=p2[:, :, a], op=mybir.AluOpType.mult)
                        nc.vector.tensor_scalar_mul(out=tmp[:], in0=tmp[:], scalar1=2.0)
                        nc.vector.tensor_tensor(out=tmp[:], in0=tmp[:], in1=acc[:], op=mybir.AluOpType.subtract)
                        nc.vector.tensor_tensor(out=tmp[:], in0=tmp[:], in1=S[:, :, fo + 3 + a], op=mybir.AluOpType.add)
                        nc.vector.tensor_tensor(out=gout[:, :, a], in0=tmp[:], in1=awb, op=mybir.AluOpType.mult)
                    nc.sync.dma_start(out=g_p[b].rearrange("(p q) c -> p q c", p=128), in_=gout[:])

    nc.compile()
    return nc


def _balance(cnt):
    """Assign vertices to 98 windows of 128 positions, balancing each window's
    per-chunk edge counts toward multiples of 128. Returns dpos[l] in [0, VPC)."""
    key = cnt @ np.array([1, 17, 17 ** 2, 17 ** 3])
    ukeys, inv = np.unique(key, return_inverse=True)
    T = len(ukeys)
    tvec = np.zeros((T, 4), np.int64)
    tvec[inv] = cnt
    avail = np.bincount(inv, minlength=T).astype(np.int64)
    order_by_type = np.argsort(inv, kind="stable")
    ptr = np.cumsum(np.bincount(inv, minlength=T))
    tv = tvec.astype(np.float64)
    dpos = np.empty(VREAL, np.int64)
    rem = VREAL
    for w in range(NW):
        npick = min(128, rem)
        if npick == 0:
            break
        t = np.full(4, npick * 4.0)
        got = np.zeros(4)
        for i in range(npick):
            m = npick - i
            ideal = (t - got) / m
            d = ((tv - ideal) ** 2).sum(1)
            d[avail == 0] = 1e18
            j = int(np.argmin(d))
            avail[j] -= 1
            ptr[j] -= 1
            dpos[order_by_type[ptr[j]]] = w * 128 + i
            got += tvec[j]
        rem -= npick
    return dpos


def _host_prep(xyz1, xyz2, neighborList, numNeighbors, accnumNeighbors,
               weightMatrix, rotations, arapWeight):
    nbr = np.asarray(neighborList).astype(np.int64)
    wm = np.asarray(weightMatrix).astype(np.float32)
    import jax.numpy as jnp

    def bf16(x):
        return np.asarray(jnp.asarray(x, jnp.bfloat16))

    # neighbor global row / token / chunk (vertex -> slot is core-independent)
    g_of_v = 12544 * (np.arange(N) // VREAL) + SLOT_OF[np.arange(N) % VREAL]
    tok_of_v = _token_of_global_row(g_of_v)

    # per-core edge arrays in (slot, k) order
    l_e = np.repeat(np.arange(VREAL), K)
    k_e = np.tile(np.arange(K), VREAL)
    s_e = SLOT_OF[l_e]

    cores = []
    dpos_list = []
    counts_all = np.zeros((NCORES, NCH, NW), np.int64)
    for r in range(NCORES):
        e_id = (VREAL * r + l_e) * K + k_e
        tok = tok_of_v[nbr[e_id]]
        ch = tok // CHTOK
        rel = tok - CHTOK * ch
        w_e = wm[e_id]
        cnt = np.zeros((VREAL, NCH), np.int64)
        np.add.at(cnt, (l_e, ch), 1)
        dpos = _balance(cnt)
        dpos_list.append(dpos)
        d_e = dpos[l_e]
        order = np.lexsort((k_e, d_e, ch))
        cores.append((ch[order], rel[order], d_e[order], w_e[order]))
        counts_all[r] = np.bincount(ch[order] * NW + d_e[order] // 128,
                                    minlength=NCH * NW).reshape(NCH, NW)

    caps = counts_all.max(axis=0)                       # [NCH, NW]
    blocks_cw = np.maximum((caps + 127) // 128, 1)      # >=1 block per cell
    nt_per_chunk = []
    for c in range(NCH):
        tot = int(blocks_cw[c].sum())
        extra = (-tot) % BPT                            # pad chunk to G-tile mult
        blocks_cw[c, NW - 1] += extra
        nt_per_chunk.append((tot + extra) // BPT)
    nt_per_chunk = tuple(nt_per_chunk)
    blocks_key = tuple(tuple(int(x) for x in blocks_cw[c]) for c in range(NCH))

    NT_TOT = sum(nt_per_chunk)
    NBLK_TOT = BPT * NT_TOT

    # cell start offsets in each chunk stream (in edges)
    cell_off = np.zeros((NCH, NW), np.int64)
    for c in range(NCH):
        cell_off[c] = np.concatenate([[0], np.cumsum(blocks_cw[c][:-1] * 128)])
    chunk_len = [int(blocks_cw[c].sum() * 128) for c in range(NCH)]
    chunk_tile_base = np.concatenate([[0], np.cumsum(nt_per_chunk)]).astype(np.int64)

    in_maps = []
    ident = np.eye(128, dtype=np.float32)
    smask_host = np.zeros(VPC, np.float32)
    smask_host[SLOT_OF] = 1.0
    smask_host = smask_host.reshape(128, QCOL)
    for r in range(NCORES):
        ch_s, rel_s, s_s, w_s = cores[r]
        idx_stream = [np.full(chunk_len[c], ZT_REL[c], np.int16) for c in range(NCH)]
        vcol_stream = [np.zeros(chunk_len[c], np.float32) for c in range(NCH)]
        wgt_stream = [np.zeros(chunk_len[c], np.float32) for c in range(NCH)]
        # place each cell's edges at its stream offset
        pos_in_cell = np.zeros(NCH * NW, np.int64)
        cell_id = ch_s * NW + s_s // 128
        # edges are sorted by (ch, slot, k) so within each cell they are in order
        # compute position of each edge within its cell:
        srt = np.argsort(cell_id, kind="stable")
        cid_sorted = cell_id[srt]
        first = np.concatenate([[0], np.cumsum(np.bincount(cid_sorted, minlength=NCH * NW))[:-1]])
        pos_sorted = np.arange(len(cid_sorted)) - first[cid_sorted]
        pos = np.empty_like(pos_sorted)
        pos[srt] = pos_sorted
        for c in range(NCH):
            m = ch_s == c
            p = cell_off[c][s_s[m] // 128] + pos[m]
            idx_stream[c][p] = rel_s[m].astype(np.int16)
            vcol_stream[c][p] = (s_s[m] % 128).astype(np.float32)
            wgt_stream[c][p] = w_s[m]

        idxw = np.zeros((NT_TOT, CPT, 128, CALL // 16), np.int16)
        xraw = np.zeros((NBLK_TOT, 128, 2), np.float32)
        for c in range(NCH):
            st = idx_stream[c].reshape(-1, CPT, CALL // 16, 16)  # [tiles, call, f, l]
            idxw[chunk_tile_base[c] : chunk_tile_base[c + 1]] = np.tile(
                np.transpose(st, (0, 1, 3, 2)), (1, 1, 8, 1))
            b0 = BPT * chunk_tile_base[c]
            nb = chunk_len[c] // 128
            xraw[b0 : b0 + nb, :, 0] = vcol_stream[c].reshape(nb, 128)
            xraw[b0 : b0 + nb, :, 1] = wgt_stream[c].reshape(nb, 128)

        in_maps.append({
            "xyz1s": None, "xyz2s": None, "rots": None,
            "idxw": idxw, "xraw": bf16(xraw), "ident": bf16(ident),
            "smask": bf16(smask_host),
            "aw": np.full((128, 1), np.float32(arapWeight)),
        })

    # permuted per-core xyz/rot inputs (slot layout, pads zero)
    xyz1 = np.asarray(xyz1)
    xyz2 = np.asarray(xyz2)
    rots = np.asarray(rotations).reshape(B, N, 9)
    for r in range(NCORES):
        x1 = np.zeros((B, VPC, 3), np.float32)
        x2 = np.zeros((B, VPC, 3), np.float32)
        rr = np.zeros((B, VPC, 9), np.float32)
        v0 = r * VREAL
        x1[:, SLOT_OF] = xyz1[:, v0 : v0 + VREAL]
        x2[:, SLOT_OF] = xyz2[:, v0 : v0 + VREAL]
        rr[:, SLOT_OF] = rots[:, v0 : v0 + VREAL]
        in_maps[r]["xyz1s"] = x1
        in_maps[r]["xyz2s"] = x2
        in_maps[r]["rots"] = rr
        x1d = np.zeros((B, VPC, 3), np.float32)
        x2d = np.zeros((B, VPC, 3), np.float32)
        rrd = np.zeros((B, VPC, 9), np.float32)
        x1d[:, dpos_list[r]] = xyz1[:, v0 : v0 + VREAL]
        x2d[:, dpos_list[r]] = xyz2[:, v0 : v0 + VREAL]
        rrd[:, dpos_list[r]] = rots[:, v0 : v0 + VREAL]
        in_maps[r]["xyz1d"] = x1d
        in_maps[r]["xyz2d"] = x2d
        in_maps[r]["rotsd"] = rrd

    return in_maps, nt_per_chunk, blocks_key, dpos_list


def _execute(in_maps, nt_per_chunk, blocks_key, trace=False, **kw):
    from concourse.bass_utils import run_bass_kernel_spmd
    key = (nt_per_chunk, blocks_key)
    if key not in _cache:
        _cache[key] = _build(nt_per_chunk, blocks_key)
    return run_bass_kernel_spmd(_cache[key], in_maps, list(range(NCORES)), trace=trace, **kw)


def kernel(**inputs):
    in_maps, ntc, bkey, dpos_list = _host_prep(**inputs)
    res = _execute(in_maps, ntc, bkey)
    out = np.concatenate(
        [res.results[r]["g"][:, dpos_list[r]] for r in range(NCORES)], axis=1)
    return out.astype(np.float32)
